# revision 9
# baseline (speedup 1.0000x reference)
"""DGCNN forward on 8 Trainium2 cores (self-contained).

500 graphs (200 nodes, block-diag edges) padded to 512, 64 graphs/core in 16
groups of 4.  Device runs GCN layers 1-3 (h1,h2,h3); the width-1 layer-4 head
(the sort key), sort, convs and MLP run on host in fp32.

Propagate streams the *raw integer* adjacency counts (exact in bf16, one pass
through the PE per layer) with the symmetric normalization factored out:

  p = inv ⊙ (A^T (inv ⊙ t)),   t = h @ W,   A = integer counts (+self loop)

src-side inv commutes with @W, so the stationary operand is the bf16 hi/lo
pair of t' = inv⊙t (t' = (inv⊙h)@W for layers 2-3, host-folded for layer 1);
dst-side inv is one elementwise multiply (IV tile, inv replicated across the
32 feature partitions) before tanh.

  transform  t' = h' @ W  4 matmuls/graph: K-stacked lhsT [h'_hi; h'_lo] with
                          rhs [W_hi; W_lo] then swapped -> all 4 cross terms
                          accumulate in psum (exact to ~2^-17).
  propagate  feat-major: lhsT = [t'_hi | t'_lo] (64 wide), stream integer
                          A^T chunk (128/72 srcs x 200 dsts) ONCE; psum rows
                          0-31/32-63 hold hi/lo partials: copy + DVE add,
                          ⊙IV (gpsimd), tanh (scalar).

Two graphs per psum tile at partition bases 0/64-style packing keeps matmul
lhsT/rhs partition bases equal (hw requirement).  Groups are processed
layer-major in two sets of 8 so independent per-group chains interleave in
the engine FIFOs.
"""
import os
import numpy as np
import ml_dtypes

N_GRAPHS, N_PER, K_TOP, F_IN, H = 500, 200, 30, 128, 32
G_PAD = 512
G_CORE = 64
NGRP = 16
NSET = 8            # groups per resident set
BF16 = ml_dtypes.bfloat16
C0, C1 = 128, 72


def _build_counts(edge_index):
    """Integer adjacency counts C[g, s, d] (src-major = matmul rhs layout),
    incl. self-loops, plus inv = deg^-1/2 per node."""
    src = edge_index[0].astype(np.int64)
    dst = edge_index[1].astype(np.int64)
    g = src // N_PER
    C = np.zeros((N_GRAPHS, N_PER, N_PER), np.float32)
    np.add.at(C, (g, src % N_PER, dst % N_PER), 1.0)
    idx = np.arange(N_PER)
    C[:, idx, idx] += 1.0
    deg = C.sum(axis=1)                      # in-degree incl self-loop
    inv = (1.0 / np.sqrt(np.maximum(deg, 1e-12))).astype(np.float32)
    return C, inv


def _host_tail(hcat, inputs):
    G = hcat.shape[0]
    order = np.argsort(-hcat[:, :, -1], axis=1, kind="stable")[:, :K_TOP]
    topk = np.take_along_axis(hcat, order[:, :, None], axis=1)
    C1w = np.asarray(inputs["cw1"], np.float32)[:, 0, :].T
    c1 = np.maximum(np.einsum("gkc,co->gko", topk, C1w) + np.asarray(inputs["cb1"], np.float32), 0)
    p1 = np.maximum(c1[:, 0::2, :], c1[:, 1::2, :])
    cw2 = np.asarray(inputs["cw2"], np.float32)
    c2 = np.zeros((G, 11, 32), np.float32)
    for k in range(5):
        c2 += np.einsum("gti,io->gto", p1[:, k:k + 11, :], cw2[:, :, k].T)
    c2 = np.maximum(c2 + np.asarray(inputs["cb2"], np.float32), 0)
    flat = c2.transpose(0, 2, 1).reshape(G, -1)
    z = np.maximum(flat @ np.asarray(inputs["lw1"], np.float32) + np.asarray(inputs["lb1"], np.float32), 0)
    o = z @ np.asarray(inputs["lw2"], np.float32) + np.asarray(inputs["lb2"], np.float32)
    return (1.0 / (1.0 + np.exp(-o))).astype(np.float32)


def _split(a):
    hi = a.astype(BF16)
    lo = (a - hi.astype(np.float32)).astype(BF16)
    return hi, lo


def _device_gcn(ins):
    import concourse.bacc as bacc
    import concourse.mybir as mybir
    import concourse.tile as tile
    from concourse import bass_utils

    dt = mybir.dt
    ACT = mybir.ActivationFunctionType
    OP = mybir.AluOpType
    nc = bacc.Bacc("TRN2", target_bir_lowering=False, debug=False, num_devices=8)

    d = {}
    for name, shape, ddt in [
        ("aX", (NGRP, 128, 800), dt.bfloat16),    # integer A^T rows 0-127
        ("aY", (NGRP, 72, 800), dt.bfloat16),     # integer A^T rows 128-199
        ("tl0", (NGRP, 128, 4, 2, 2, 32), dt.bfloat16),  # host t1' = inv*(x@W1) pair
        ("iv", (NGRP, 64, 2, 200), dt.float32),   # inv_dst replicated over feats
        ("w2r", (64, 32), dt.bfloat16), ("w2s", (64, 32), dt.bfloat16),
        ("w3r", (64, 32), dt.bfloat16), ("w3s", (64, 32), dt.bfloat16),
    ]:
        d[name] = nc.dram_tensor(name, shape, ddt, kind="ExternalInput").ap()
    # H: [grp, layer, pr*32feat, par, node]
    d_H = nc.dram_tensor("H", (NGRP, 3, 64, 2, 200), dt.float32, kind="ExternalOutput").ap()

    with tile.TileContext(nc) as tc:
        with tc.tile_pool(name="wp", bufs=1) as wp, \
             tc.tile_pool(name="ain", bufs=2) as ain, \
             tc.tile_pool(name="ainx", bufs=2) as ainx, \
             tc.tile_pool(name="ivp", bufs=2) as ivp, \
             tc.tile_pool(name="sb", bufs=1) as sb, \
             tc.tile_pool(name="hhp", bufs=1) as hhp, \
             tc.tile_pool(name="tlp", bufs=2) as tlp, \
             tc.tile_pool(name="pst", bufs=2, space="PSUM") as pst, \
             tc.tile_pool(name="psp", bufs=3, space="PSUM") as psp:
            W = {}
            for name in ["w2r", "w2s", "w3r", "w3s"]:
                W[name] = wp.tile([64, 32], dt.bfloat16, name=name)
                nc.sync.dma_start(out=W[name][:], in_=d[name])
            WR = [None, (W["w2r"], W["w2s"]), (W["w3r"], W["w3s"])]

            for st in range(NGRP // NSET):
                AX, AY, XT, IVt, hh_prev = {}, {}, {}, {}, {}
                for sl in range(NSET):
                    grp = st * NSET + sl
                    AX[sl] = ain.tile([128, 800], dt.bfloat16, tag=f"aX{sl}", name=f"aX{sl}")
                    nc.sync.dma_start(out=AX[sl][:], in_=d["aX"][grp])
                    AY[sl] = ain.tile([72, 800], dt.bfloat16, tag=f"aY{sl}", name=f"aY{sl}")
                    nc.sync.dma_start(out=AY[sl][:], in_=d["aY"][grp])
                    XT[sl] = ainx.tile([128, 4, 2, 2, 32], dt.bfloat16, tag=f"tl0{sl}", name=f"tl0{sl}")
                    nc.sync.dma_start(out=XT[sl][:], in_=d["tl0"][grp])
                    IVt[sl] = ivp.tile([64, 2, 200], dt.float32, tag=f"iv{sl}", name=f"iv{sl}")
                    nc.sync.dma_start(out=IVt[sl][:], in_=d["iv"][grp])
                # phase-major emission: each engine's FIFO sees producers
                # many slots ahead of consumers -> no head-of-line stalls
                for l in range(3):
                    TLs, T2s, p2s, Ms, MVs, HTs, HVs = {}, {}, {}, {}, {}, {}, {}
                    if l >= 1:
                        wr, ws = WR[l]
                        for sl in range(NSET):
                            T2 = pst.tile([128, 4, 2, 32], dt.float32, tag="T2", name="T2")
                            for pr in range(2):
                                hh = hh_prev[(sl, pr)]
                                for par in range(2):
                                    g = 2 * pr + par
                                    for c, cn in ((0, C0), (1, C1)):
                                        out = T2[0:cn, g, c, :]
                                        hsl = hh[0:64, par, c * 128:c * 128 + cn]
                                        nc.tensor.matmul(out, lhsT=hsl, rhs=wr[:],
                                                         start=True, stop=False)
                                        nc.tensor.matmul(out, lhsT=hsl, rhs=ws[:],
                                                         start=False, stop=True)
                            T2s[sl] = T2
                            # split t' -> bf16 pair  TL[p, g, c, hi/lo, f]
                            TL = tlp.tile([128, 4, 2, 2, 32], dt.bfloat16, tag="TL", name="TL")
                            nc.scalar.activation(TL[:, :, :, 0, :], T2[:, :, :, :], ACT.Copy)
                            TLs[sl] = TL
                        for sl in range(NSET):
                            nc.vector.tensor_tensor(TLs[sl][:, :, :, 1, :], T2s[sl][:, :, :, :], TLs[sl][:, :, :, 0, :], OP.subtract)
                    for sl in range(NSET):
                        p2 = {}
                        for pr in range(2):
                            p2[pr] = psp.tile([64, 2, 200], dt.float32, tag=f"p2{pr}", name=f"p2{pr}")
                            for par in range(2):
                                g = 2 * pr + par
                                gc = slice(200 * g, 200 * g + 200)
                                if l == 0:
                                    lh0 = XT[sl][:, g, 0]
                                    lh1 = XT[sl][0:72, g, 1]
                                else:
                                    lh0 = TLs[sl][:, g, 0]
                                    lh1 = TLs[sl][0:72, g, 1]
                                pout = p2[pr][0:64, par, :]
                                nc.tensor.matmul(pout, lhsT=lh0, rhs=AX[sl][:, gc],
                                                 start=True, stop=False)
                                nc.tensor.matmul(pout, lhsT=lh1, rhs=AY[sl][0:72, gc],
                                                 start=False, stop=True)
                        p2s[sl] = p2
                    # pair-sum halves: psum -> sbuf copy then add
                    tcs = {}
                    for sl in range(NSET):
                        for pr in range(2):
                            tmpc = sb.tile([32, 2, 200], dt.float32, tag=f"tc{sl}{pr}", name=f"tc{sl}{pr}")
                            if pr == 0:
                                nc.scalar.activation(tmpc[:, :, :], p2s[sl][pr][32:64, :, :], ACT.Copy)
                            else:
                                nc.vector.tensor_copy(tmpc[:, :, :], p2s[sl][pr][32:64, :, :])
                            tcs[(sl, pr)] = tmpc
                    for sl in range(NSET):
                        M = sb.tile([64, 2, 200], dt.float32, tag=f"M{sl}", name=f"M{sl}")
                        for pr in range(2):
                            b = 32 * pr
                            nc.vector.tensor_tensor(M[b:b + 32, :, :], p2s[sl][pr][0:32, :, :], tcs[(sl, pr)][:, :, :], OP.add)
                        Ms[sl] = M
                    for sl in range(NSET):
                        MV = sb.tile([64, 2, 200], dt.float32, tag=f"MV{sl}", name=f"MV{sl}")
                        nc.gpsimd.tensor_tensor(MV[:, :, :], Ms[sl][:, :, :], IVt[sl][:, :, :], OP.mult)
                        MVs[sl] = MV
                    for sl in range(NSET):
                        grp = st * NSET + sl
                        HT = sb.tile([64, 2, 200], dt.float32, tag=f"HT{sl}", name=f"HT{sl}")
                        nc.scalar.activation(HT[:, :, :], MVs[sl][:, :, :], ACT.Tanh)
                        nc.sync.dma_start(out=d_H[grp, l], in_=HT[:])
                        HTs[sl] = HT
                    if l < 2:
                        # src-scale for the next transform: h' = inv ⊙ h
                        for sl in range(NSET):
                            HV = sb.tile([64, 2, 200], dt.float32, tag=f"HV{sl}", name=f"HV{sl}")
                            nc.gpsimd.tensor_tensor(HV[:, :, :], HTs[sl][:, :, :], IVt[sl][:, :, :], OP.mult)
                            HVs[sl] = HV
                        # split h' -> bf16 pair, one tile per pr (lhsT base-0 rule)
                        for sl in range(NSET):
                            hh0 = hhp.tile([64, 2, 200], dt.bfloat16, tag=f"hh{sl}0", name=f"hh{sl}0")
                            nc.scalar.activation(hh0[0:32, :, :], HVs[sl][0:32, :, :], ACT.Copy)
                            hh1 = hhp.tile([64, 2, 200], dt.bfloat16, tag=f"hh{sl}1", name=f"hh{sl}1")
                            nc.scalar.activation(hh1[32:64, :, :], HVs[sl][32:64, :, :], ACT.Copy)
                            hh_prev[(sl, 0)] = hh0
                            hh_prev[(sl, 1)] = hh1
                        for sl in range(NSET):
                            nc.vector.tensor_tensor(hh_prev[(sl, 0)][32:64, :, :], HVs[sl][0:32, :, :], hh_prev[(sl, 0)][0:32, :, :], OP.subtract)
                            nc.gpsimd.tensor_tensor(hh_prev[(sl, 1)][0:32, :, :], HVs[sl][32:64, :, :], hh_prev[(sl, 1)][32:64, :, :], OP.subtract)

    nc.compile()

    trace = bool(int(os.environ.get("BASS_KERNEL_TRACE", "0")))
    res = bass_utils.run_bass_kernel_spmd(nc, ins, core_ids=list(range(8)), trace=trace)
    if trace and res.exec_time_ns is not None:
        print(f"HW exec time: {res.exec_time_ns} ns")
    return res.results


def kernel(**inputs):
    x = np.asarray(inputs["x"], np.float32)
    ei = np.asarray(inputs["edge_index"])
    C, inv = _build_counts(ei)
    Ws = [np.asarray(inputs[f"W{i}"], np.float32) for i in (1, 2, 3, 4)]
    bs = [np.asarray(inputs[f"b{i}"], np.float32) for i in (1, 2, 3, 4)]
    xg = x.reshape(N_GRAPHS, N_PER, F_IN)

    use_device = all(np.all(b == 0) for b in bs) and C.max() <= 256
    hcat = None
    if use_device:
        try:
            Cp = np.zeros((G_PAD, N_PER, N_PER), np.float32)
            Cp[:N_GRAPHS] = C
            invp = np.ones((G_PAD, N_PER), np.float32)
            invp[:N_GRAPHS] = inv
            Chi = Cp.astype(BF16)        # exact: integer counts <= 256

            t1 = np.einsum("gnf,fo->gno", xg.astype(np.float64), Ws[0].astype(np.float64), optimize=True).astype(np.float32)
            t1p = np.zeros((G_PAD, N_PER, 32), np.float32)
            t1p[:N_GRAPHS] = t1 * inv[:, :, None]
            t1hi, t1lo = _split(t1p)
            # TL layout [p, c, hi/lo, f] per graph
            TLh = np.zeros((G_PAD, 128, 2, 2, 32), BF16)
            TLh[:, :, 0, 0] = t1hi[:, 0:128]
            TLh[:, :, 0, 1] = t1lo[:, 0:128]
            TLh[:, 0:72, 1, 0] = t1hi[:, 128:200]
            TLh[:, 0:72, 1, 1] = t1lo[:, 128:200]

            def core_view(arr, rows):
                # arr [512, rows, 200] bf16 -> [8, NGRP, rows, 800]
                return (arr.reshape(8, NGRP, 4, rows, 200)
                           .transpose(0, 1, 3, 2, 4).reshape(8, NGRP, rows, 800).copy())

            aX = core_view(np.ascontiguousarray(Chi[:, 0:128]), 128)   # [8,16,128,800]
            aY = core_view(np.ascontiguousarray(Chi[:, 128:200]), 72)  # [8,16,72,800]
            tl0 = (TLh.reshape(8, NGRP, 4, 128, 2, 2, 32)
                      .transpose(0, 1, 3, 2, 4, 5, 6).copy())          # [8,16,128,4,2,2,32]

            invc = invp.reshape(8, NGRP, 4, N_PER)
            IV = np.empty((8, NGRP, 64, 2, N_PER), np.float32)
            for pr in (0, 1):
                for par in (0, 1):
                    IV[:, :, 32 * pr:32 * pr + 32, par, :] = invc[:, :, None, 2 * pr + par, :]

            def wpair(Wm):
                hi, lo = _split(Wm)
                pair = np.concatenate([hi.astype(np.float32), lo.astype(np.float32)], axis=0)
                swap = np.concatenate([lo.astype(np.float32), hi.astype(np.float32)], axis=0)
                return pair.astype(BF16), swap.astype(BF16)

            w2r, w2s = wpair(Ws[1])
            w3r, w3s = wpair(Ws[2])

            ins = [{"aX": aX[c], "aY": aY[c], "tl0": tl0[c], "iv": IV[c],
                    "w2r": w2r, "w2s": w2s, "w3r": w3r, "w3s": w3s} for c in range(8)]
            res = _device_gcn(ins)

            hs = []
            for l in range(3):
                v = np.stack([res[c]["H"][:, l] for c in range(8)])   # [8,16,64,2par,200]
                v = v.reshape(8, NGRP, 2, 32, 2, 200)                  # [.., pr, feat, par, node]
                v = v.transpose(0, 1, 2, 4, 5, 3)                      # [.., pr, par, node, feat]
                hs.append(v.reshape(G_PAD, N_PER, 32)[:N_GRAPHS])
            # width-1 head (the sort key) in full fp32 on host
            h3 = hs[2]
            t4 = (h3.reshape(-1, 32) @ Ws[3]).reshape(N_GRAPHS, N_PER)
            p4 = np.einsum("gsd,gs->gd", C, t4 * inv, optimize=True)
            h4 = np.tanh(inv * p4)[:, :, None]
            hcat = np.concatenate(hs + [h4], axis=-1)
        except Exception as e:
            print("device path failed, falling back to host:", repr(e))
            hcat = None
    if hcat is None:
        # host fallback: dense normalized adjacency per graph
        An = C * inv[:, :, None] * inv[:, None, :]   # [g, s, d] normalized
        h = xg
        hs = []
        for l in range(4):
            h = np.tanh(np.einsum("gsd,gsf->gdf", An, h) @ Ws[l] + bs[l])
            hs.append(h)
        hcat = np.concatenate(hs, axis=-1)
    return _host_tail(hcat, inputs)


# revision 10
# speedup vs baseline: 1.0300x; 1.0300x over previous
"""DGCNN forward on 8 Trainium2 cores (self-contained).

500 graphs (200 nodes, block-diag edges) padded to 512, 64 graphs/core in 16
groups of 4.  Device runs GCN layers 1-3 (h1,h2,h3); the width-1 layer-4 head
(the sort key), sort, convs and MLP run on host in fp32.

Propagate streams the *raw integer* adjacency counts (exact in bf16, one pass
through the PE per layer) with the symmetric normalization factored out:

  p = inv ⊙ (A^T (inv ⊙ t)),   t = h @ W,   A = integer counts (+self loop)

src-side inv commutes with @W, so the stationary operand is the bf16 hi/lo
pair of t' = inv⊙t (t' = (inv⊙h)@W for layers 2-3, host-folded for layer 1);
dst-side inv is one elementwise multiply (IV tile, inv replicated across the
32 feature partitions) before tanh.

  transform  t' = h' @ W  4 matmuls/graph: K-stacked lhsT [h'_hi; h'_lo] with
                          rhs [W_hi; W_lo] then swapped -> all 4 cross terms
                          accumulate in psum (exact to ~2^-17).
  propagate  feat-major: lhsT = [t'_hi | t'_lo] (64 wide), stream integer
                          A^T chunk (128/72 srcs x 200 dsts) ONCE; psum rows
                          0-31/32-63 hold hi/lo partials: copy + DVE add,
                          ⊙IV (gpsimd), tanh (scalar).

Two graphs per psum tile at partition bases 0/64-style packing keeps matmul
lhsT/rhs partition bases equal (hw requirement).  Groups are processed
layer-major in two sets of 8 so independent per-group chains interleave in
the engine FIFOs.
"""
import os
import numpy as np
import ml_dtypes

N_GRAPHS, N_PER, K_TOP, F_IN, H = 500, 200, 30, 128, 32
G_PAD = 512
G_CORE = 64
NGRP = 16
NSET = 8            # groups per resident set
BF16 = ml_dtypes.bfloat16
C0, C1 = 128, 72


def _build_counts(edge_index):
    """Integer adjacency counts C[g, s, d] (src-major = matmul rhs layout),
    incl. self-loops, plus inv = deg^-1/2 per node."""
    src = edge_index[0].astype(np.int64)
    dst = edge_index[1].astype(np.int64)
    g = src // N_PER
    C = np.zeros((N_GRAPHS, N_PER, N_PER), np.float32)
    np.add.at(C, (g, src % N_PER, dst % N_PER), 1.0)
    idx = np.arange(N_PER)
    C[:, idx, idx] += 1.0
    deg = C.sum(axis=1)                      # in-degree incl self-loop
    inv = (1.0 / np.sqrt(np.maximum(deg, 1e-12))).astype(np.float32)
    return C, inv


def _host_tail(hcat, inputs):
    G = hcat.shape[0]
    order = np.argsort(-hcat[:, :, -1], axis=1, kind="stable")[:, :K_TOP]
    topk = np.take_along_axis(hcat, order[:, :, None], axis=1)
    C1w = np.asarray(inputs["cw1"], np.float32)[:, 0, :].T
    c1 = np.maximum(np.einsum("gkc,co->gko", topk, C1w) + np.asarray(inputs["cb1"], np.float32), 0)
    p1 = np.maximum(c1[:, 0::2, :], c1[:, 1::2, :])
    cw2 = np.asarray(inputs["cw2"], np.float32)
    c2 = np.zeros((G, 11, 32), np.float32)
    for k in range(5):
        c2 += np.einsum("gti,io->gto", p1[:, k:k + 11, :], cw2[:, :, k].T)
    c2 = np.maximum(c2 + np.asarray(inputs["cb2"], np.float32), 0)
    flat = c2.transpose(0, 2, 1).reshape(G, -1)
    z = np.maximum(flat @ np.asarray(inputs["lw1"], np.float32) + np.asarray(inputs["lb1"], np.float32), 0)
    o = z @ np.asarray(inputs["lw2"], np.float32) + np.asarray(inputs["lb2"], np.float32)
    return (1.0 / (1.0 + np.exp(-o))).astype(np.float32)


def _split(a):
    hi = a.astype(BF16)
    lo = (a - hi.astype(np.float32)).astype(BF16)
    return hi, lo


def _device_gcn(ins):
    import concourse.bacc as bacc
    import concourse.mybir as mybir
    import concourse.tile as tile
    from concourse import bass_utils

    dt = mybir.dt
    ACT = mybir.ActivationFunctionType
    OP = mybir.AluOpType
    nc = bacc.Bacc("TRN2", target_bir_lowering=False, debug=False, num_devices=8)

    d = {}
    for name, shape, ddt in [
        ("aX", (NGRP, 128, 800), dt.bfloat16),    # integer A^T rows 0-127
        ("aY", (NGRP, 72, 800), dt.bfloat16),     # integer A^T rows 128-199
        ("tl0", (NGRP, 128, 4, 2, 2, 32), dt.bfloat16),  # host t1' = inv*(x@W1) pair
        ("iv", (NGRP, 64, 2, 200), dt.float32),   # inv_dst replicated over feats
        ("w2r", (64, 32), dt.bfloat16), ("w2s", (64, 32), dt.bfloat16),
        ("w3r", (64, 32), dt.bfloat16), ("w3s", (64, 32), dt.bfloat16),
    ]:
        d[name] = nc.dram_tensor(name, shape, ddt, kind="ExternalInput").ap()
    # H: [grp, layer, pr*32feat, par, node]
    d_H = nc.dram_tensor("H", (NGRP, 3, 64, 2, 200), dt.float32, kind="ExternalOutput").ap()

    with tile.TileContext(nc) as tc:
        with tc.tile_pool(name="wp", bufs=1) as wp, \
             tc.tile_pool(name="ain", bufs=2) as ain, \
             tc.tile_pool(name="ainx", bufs=2) as ainx, \
             tc.tile_pool(name="ivp", bufs=2) as ivp, \
             tc.tile_pool(name="sb", bufs=1) as sb, \
             tc.tile_pool(name="hhp", bufs=1) as hhp, \
             tc.tile_pool(name="tlp", bufs=2) as tlp, \
             tc.tile_pool(name="pst", bufs=2, space="PSUM") as pst, \
             tc.tile_pool(name="psp", bufs=3, space="PSUM") as psp:
            W = {}
            for name in ["w2r", "w2s", "w3r", "w3s"]:
                W[name] = wp.tile([64, 32], dt.bfloat16, name=name)
                nc.sync.dma_start(out=W[name][:], in_=d[name])
            WR = [None, (W["w2r"], W["w2s"]), (W["w3r"], W["w3s"])]

            for st in range(NGRP // NSET):
                AX, AY, XT, IVt, hh_prev = {}, {}, {}, {}, {}
                for sl in range(NSET):
                    grp = st * NSET + sl
                    AX[sl] = ain.tile([128, 800], dt.bfloat16, tag=f"aX{sl}", name=f"aX{sl}")
                    nc.sync.dma_start(out=AX[sl][:], in_=d["aX"][grp])
                    AY[sl] = ain.tile([72, 800], dt.bfloat16, tag=f"aY{sl}", name=f"aY{sl}")
                    nc.sync.dma_start(out=AY[sl][:], in_=d["aY"][grp])
                    XT[sl] = ainx.tile([128, 4, 2, 2, 32], dt.bfloat16, tag=f"tl0{sl}", name=f"tl0{sl}")
                    nc.sync.dma_start(out=XT[sl][:], in_=d["tl0"][grp])
                    IVt[sl] = ivp.tile([64, 2, 200], dt.float32, tag=f"iv{sl}", name=f"iv{sl}")
                    nc.sync.dma_start(out=IVt[sl][:], in_=d["iv"][grp])
                # phase-major emission: each engine's FIFO sees producers
                # many slots ahead of consumers -> no head-of-line stalls
                for l in range(3):
                    TLs, T2s, p2s, Ms, MVs, HTs, HVs = {}, {}, {}, {}, {}, {}, {}
                    if l >= 1:
                        wr, ws = WR[l]
                        for sl in range(NSET):
                            T2 = pst.tile([128, 4, 2, 32], dt.float32, tag="T2", name="T2")
                            for pr in range(2):
                                hh = hh_prev[(sl, pr)]
                                for par in range(2):
                                    g = 2 * pr + par
                                    for c, cn in ((0, C0), (1, C1)):
                                        out = T2[0:cn, g, c, :]
                                        hsl = hh[0:64, par, c * 128:c * 128 + cn]
                                        nc.tensor.matmul(out, lhsT=hsl, rhs=wr[:],
                                                         start=True, stop=False)
                                        nc.tensor.matmul(out, lhsT=hsl, rhs=ws[:],
                                                         start=False, stop=True)
                            T2s[sl] = T2
                            # split t' -> bf16 pair  TL[p, g, c, hi/lo, f]
                            TL = tlp.tile([128, 4, 2, 2, 32], dt.bfloat16, tag="TL", name="TL")
                            nc.scalar.activation(TL[:, :, :, 0, :], T2[:, :, :, :], ACT.Copy)
                            TLs[sl] = TL
                        for sl in range(NSET):
                            nc.vector.tensor_tensor(TLs[sl][:, :, :, 1, :], T2s[sl][:, :, :, :], TLs[sl][:, :, :, 0, :], OP.subtract)
                    for sl in range(NSET):
                        p2 = {}
                        for pr in range(2):
                            p2[pr] = psp.tile([64, 2, 200], dt.float32, tag=f"p2{pr}", name=f"p2{pr}")
                            for par in range(2):
                                g = 2 * pr + par
                                gc = slice(200 * g, 200 * g + 200)
                                if l == 0:
                                    lh0 = XT[sl][:, g, 0]
                                    lh1 = XT[sl][0:72, g, 1]
                                else:
                                    lh0 = TLs[sl][:, g, 0]
                                    lh1 = TLs[sl][0:72, g, 1]
                                pout = p2[pr][0:64, par, :]
                                nc.tensor.matmul(pout, lhsT=lh0, rhs=AX[sl][:, gc],
                                                 start=True, stop=False)
                                nc.tensor.matmul(pout, lhsT=lh1, rhs=AY[sl][0:72, gc],
                                                 start=False, stop=True)
                        p2s[sl] = p2
                    # pair-sum halves: psum -> sbuf copy then add
                    tcs = {}
                    for sl in range(NSET):
                        for pr in range(2):
                            tmpc = sb.tile([32, 2, 200], dt.float32, tag=f"tc{sl}{pr}", name=f"tc{sl}{pr}")
                            if pr == 0:
                                nc.scalar.activation(tmpc[:, :, :], p2s[sl][pr][32:64, :, :], ACT.Copy)
                            else:
                                nc.vector.tensor_copy(tmpc[:, :, :], p2s[sl][pr][32:64, :, :])
                            tcs[(sl, pr)] = tmpc
                    for sl in range(NSET):
                        M = sb.tile([64, 2, 200], dt.float32, tag=f"M{sl}", name=f"M{sl}")
                        for pr in range(2):
                            b = 32 * pr
                            nc.vector.tensor_tensor(M[b:b + 32, :, :], p2s[sl][pr][0:32, :, :], tcs[(sl, pr)][:, :, :], OP.add)
                        Ms[sl] = M
                    # tail: interleaved per-group chains so next layer's
                    # transforms see early groups' hh ready quickly
                    for sl in range(NSET):
                        grp = st * NSET + sl
                        MV = sb.tile([64, 2, 200], dt.float32, tag=f"MV{sl}", name=f"MV{sl}")
                        nc.gpsimd.tensor_tensor(MV[:, :, :], Ms[sl][:, :, :], IVt[sl][:, :, :], OP.mult)
                        HT = sb.tile([64, 2, 200], dt.float32, tag=f"HT{sl}", name=f"HT{sl}")
                        nc.scalar.activation(HT[:, :, :], MV[:, :, :], ACT.Tanh)
                        nc.sync.dma_start(out=d_H[grp, l], in_=HT[:])
                        if l < 2:
                            # src-scale for the next transform: h' = inv ⊙ h
                            HV = sb.tile([64, 2, 200], dt.float32, tag=f"HV{sl}", name=f"HV{sl}")
                            nc.gpsimd.tensor_tensor(HV[:, :, :], HT[:, :, :], IVt[sl][:, :, :], OP.mult)
                            # split h' -> bf16 pair, one tile per pr (lhsT base-0 rule)
                            hh0 = hhp.tile([64, 2, 200], dt.bfloat16, tag=f"hh{sl}0", name=f"hh{sl}0")
                            nc.scalar.activation(hh0[0:32, :, :], HV[0:32, :, :], ACT.Copy)
                            nc.vector.tensor_tensor(hh0[32:64, :, :], HV[0:32, :, :], hh0[0:32, :, :], OP.subtract)
                            hh1 = hhp.tile([64, 2, 200], dt.bfloat16, tag=f"hh{sl}1", name=f"hh{sl}1")
                            nc.scalar.activation(hh1[32:64, :, :], HV[32:64, :, :], ACT.Copy)
                            nc.gpsimd.tensor_tensor(hh1[0:32, :, :], HV[32:64, :, :], hh1[32:64, :, :], OP.subtract)
                            hh_prev[(sl, 0)] = hh0
                            hh_prev[(sl, 1)] = hh1

    nc.compile()

    trace = bool(int(os.environ.get("BASS_KERNEL_TRACE", "0")))
    res = bass_utils.run_bass_kernel_spmd(nc, ins, core_ids=list(range(8)), trace=trace)
    if trace and res.exec_time_ns is not None:
        print(f"HW exec time: {res.exec_time_ns} ns")
    return res.results


def kernel(**inputs):
    x = np.asarray(inputs["x"], np.float32)
    ei = np.asarray(inputs["edge_index"])
    C, inv = _build_counts(ei)
    Ws = [np.asarray(inputs[f"W{i}"], np.float32) for i in (1, 2, 3, 4)]
    bs = [np.asarray(inputs[f"b{i}"], np.float32) for i in (1, 2, 3, 4)]
    xg = x.reshape(N_GRAPHS, N_PER, F_IN)

    use_device = all(np.all(b == 0) for b in bs) and C.max() <= 256
    hcat = None
    if use_device:
        try:
            Cp = np.zeros((G_PAD, N_PER, N_PER), np.float32)
            Cp[:N_GRAPHS] = C
            invp = np.ones((G_PAD, N_PER), np.float32)
            invp[:N_GRAPHS] = inv
            Chi = Cp.astype(BF16)        # exact: integer counts <= 256

            t1 = np.einsum("gnf,fo->gno", xg.astype(np.float64), Ws[0].astype(np.float64), optimize=True).astype(np.float32)
            t1p = np.zeros((G_PAD, N_PER, 32), np.float32)
            t1p[:N_GRAPHS] = t1 * inv[:, :, None]
            t1hi, t1lo = _split(t1p)
            # TL layout [p, c, hi/lo, f] per graph
            TLh = np.zeros((G_PAD, 128, 2, 2, 32), BF16)
            TLh[:, :, 0, 0] = t1hi[:, 0:128]
            TLh[:, :, 0, 1] = t1lo[:, 0:128]
            TLh[:, 0:72, 1, 0] = t1hi[:, 128:200]
            TLh[:, 0:72, 1, 1] = t1lo[:, 128:200]

            def core_view(arr, rows):
                # arr [512, rows, 200] bf16 -> [8, NGRP, rows, 800]
                return (arr.reshape(8, NGRP, 4, rows, 200)
                           .transpose(0, 1, 3, 2, 4).reshape(8, NGRP, rows, 800).copy())

            aX = core_view(np.ascontiguousarray(Chi[:, 0:128]), 128)   # [8,16,128,800]
            aY = core_view(np.ascontiguousarray(Chi[:, 128:200]), 72)  # [8,16,72,800]
            tl0 = (TLh.reshape(8, NGRP, 4, 128, 2, 2, 32)
                      .transpose(0, 1, 3, 2, 4, 5, 6).copy())          # [8,16,128,4,2,2,32]

            invc = invp.reshape(8, NGRP, 4, N_PER)
            IV = np.empty((8, NGRP, 64, 2, N_PER), np.float32)
            for pr in (0, 1):
                for par in (0, 1):
                    IV[:, :, 32 * pr:32 * pr + 32, par, :] = invc[:, :, None, 2 * pr + par, :]

            def wpair(Wm):
                hi, lo = _split(Wm)
                pair = np.concatenate([hi.astype(np.float32), lo.astype(np.float32)], axis=0)
                swap = np.concatenate([lo.astype(np.float32), hi.astype(np.float32)], axis=0)
                return pair.astype(BF16), swap.astype(BF16)

            w2r, w2s = wpair(Ws[1])
            w3r, w3s = wpair(Ws[2])

            ins = [{"aX": aX[c], "aY": aY[c], "tl0": tl0[c], "iv": IV[c],
                    "w2r": w2r, "w2s": w2s, "w3r": w3r, "w3s": w3s} for c in range(8)]
            res = _device_gcn(ins)

            hs = []
            for l in range(3):
                v = np.stack([res[c]["H"][:, l] for c in range(8)])   # [8,16,64,2par,200]
                v = v.reshape(8, NGRP, 2, 32, 2, 200)                  # [.., pr, feat, par, node]
                v = v.transpose(0, 1, 2, 4, 5, 3)                      # [.., pr, par, node, feat]
                hs.append(v.reshape(G_PAD, N_PER, 32)[:N_GRAPHS])
            # width-1 head (the sort key) in full fp32 on host
            h3 = hs[2]
            t4 = (h3.reshape(-1, 32) @ Ws[3]).reshape(N_GRAPHS, N_PER)
            p4 = np.einsum("gsd,gs->gd", C, t4 * inv, optimize=True)
            h4 = np.tanh(inv * p4)[:, :, None]
            hcat = np.concatenate(hs + [h4], axis=-1)
        except Exception as e:
            print("device path failed, falling back to host:", repr(e))
            hcat = None
    if hcat is None:
        # host fallback: dense normalized adjacency per graph
        An = C * inv[:, :, None] * inv[:, None, :]   # [g, s, d] normalized
        h = xg
        hs = []
        for l in range(4):
            h = np.tanh(np.einsum("gsd,gsf->gdf", An, h) @ Ws[l] + bs[l])
            hs.append(h)
        hcat = np.concatenate(hs, axis=-1)
    return _host_tail(hcat, inputs)
